# revision 1
# baseline (speedup 1.0000x reference)
"""DSAttention TRN2 Bass kernel.

Reference (per batch b, head h, branch):
    z[l,s] = (q[l]·k[s]) * tau[b]/8 + delta[b,s]/8        (causal: s <= l)
    A = softmax_s(z);  O = A @ V
    out = m*O_edit + (1-m)*O_null,  m = soft_mask[b,l]

Sharding: B*H = 16 (b,h) slices -> 8 cores x 2 heads. Same SPMD program on
every core; core c gets b = c//4, heads 2*(c%4), 2*(c%4)+1.

Per-core algorithm (transposed-score flash attention):
  - Host pre-packs per head: qt = [Q^T; Q^T] (dup) and kt = [K_e^T; K_n^T]
    [128, L] so the two branches' QK^T matmuls run row-packed
    (tile_position (0,0)/(64,0)) concurrently on the PE array, and V with a
    ones column appended ([S, 65]) in natural layout.
  - scores^T tile [s:128, l:<=512] per (S-tile, L-chunk), trimmed to the
    causal region; exp on ACT with fused scale=tau/8 and bias=delta_s/8
    (per-partition APs), both branches in one ACTIVATE; diagonal 128x128
    block masked post-exp with a host-provided triangular 0/1 tile (DVE).
  - O^T[65, 512] accumulates AV matmuls over S-tiles (V augmented with the
    ones column => row 64 of O^T is the softmax denominator).
  - epilogue: PE-transpose O^T -> [l:128, 65], per-partition reciprocal of
    the denominator, blend branches with soft_mask, DMA out.
  - matmuls run in float32r (single-pass fp32, 1 cyc/row at N>=256).

REPEAT > 1 wraps the whole per-core program in a hardware For_i loop; used
by the timing harness to measure per-iteration HW time from wall-clock
deltas (transfers cancel).
"""

import contextlib

import numpy as np

import concourse.bass as bass
import concourse.tile as tile
from concourse import bacc, mybir
from concourse.bass_utils import run_bass_kernel_spmd

B, L, S, H, E, D = 2, 2048, 2048, 8, 64, 64
NCORES = 8
HPC = 2            # heads per core
NT = 16            # 128-row tiles in 2048
LCH = 4            # 512-wide L chunks
F32 = mybir.dt.float32
F32R = mybir.dt.float32r
EXPF = mybir.ActivationFunctionType.Exp
MUL = mybir.AluOpType.mult
ADD = mybir.AluOpType.add

TRACE = False
LAST_EXEC_NS = None
PTS_BUFS = 6
OSB_BUFS = 3
OB_BUFS = 4
REPEAT = 1
INTERLEAVE = False
MASK_GPS = False
COPY_GPS = False
JS_DESC = False
HALF_LOADS = False

_NC = None


def _build():
    nc = bacc.Bacc("TRN2")
    qt_p = nc.declare_dram_parameter("qt", [HPC, 128, L], F32R, isOutput=False)
    kt_p = nc.declare_dram_parameter("kt", [HPC, 128, S], F32R, isOutput=False)
    v_p = nc.declare_dram_parameter("v", [HPC, S, D + 1], F32R, isOutput=False)
    vn_p = nc.declare_dram_parameter("vn", [HPC, S, D + 1], F32R, isOutput=False)
    st_p = nc.declare_dram_parameter("st", [128, 1], F32, isOutput=False)
    cd_p = nc.declare_dram_parameter("cdelta", [128, NT], F32, isOutput=False)
    mt_p = nc.declare_dram_parameter("mt", [128, NT], F32, isOutput=False)
    mt1_p = nc.declare_dram_parameter("mt1", [128, NT], F32, isOutput=False)
    id_p = nc.declare_dram_parameter("ident", [128, 128], F32, isOutput=False)
    mk_p = nc.declare_dram_parameter("mask", [128, 128], F32R, isOutput=False)
    out_p = nc.declare_dram_parameter("out", [HPC, L, D], F32, isOutput=True)
    params = (qt_p, kt_p, v_p, vn_p, st_p, cd_p, mt_p, mt1_p, id_p, mk_p, out_p)

    with tile.TileContext(nc) as tc:
        with (
            tc.tile_pool(name="const", bufs=1) as const,
            tc.tile_pool(name="big", bufs=2) as big,
            tc.tile_pool(name="pts", bufs=PTS_BUFS) as pts,
            tc.tile_pool(name="osb", bufs=OSB_BUFS) as osb,
            tc.tile_pool(name="sml", bufs=8) as sml,
            tc.tile_pool(name="ob", bufs=OB_BUFS) as ob,
            tc.tile_pool(name="ps_pt", bufs=2, space="PSUM") as ps_pt,
            tc.tile_pool(name="ps_oac", bufs=1, space="PSUM") as ps_oac,
            tc.tile_pool(name="ps_tr", bufs=2, space="PSUM") as ps_tr,
        ):
            pools = (const, big, pts, osb, sml, ob, ps_pt, ps_oac, ps_tr)
            rep = (
                tc.For_i(0, REPEAT, 1)
                if REPEAT > 1
                else contextlib.nullcontext()
            )
            with rep:
                _body(nc, pools, params)
    if not nc.is_finalized():
        nc.finalize()
    return nc


def _body(nc, pools, params):
    const, big, pts, osb, sml, ob, ps_pt, ps_oac, ps_tr = pools
    qt_p, kt_p, v_p, vn_p, st_p, cd_p, mt_p, mt1_p, id_p, mk_p, out_p = params

    ident = const.tile([128, 128], F32, tag="ident")
    nc.sync.dma_start(out=ident, in_=id_p[:])
    mask = const.tile([128, 128], F32R, tag="mask")
    nc.sync.dma_start(out=mask, in_=mk_p[:])
    cdelta = const.tile([128, NT], F32, tag="cdelta")
    nc.sync.dma_start(out=cdelta, in_=cd_p[:])
    mt = const.tile([128, NT], F32, tag="mt")
    nc.sync.dma_start(out=mt, in_=mt_p[:])
    mt1 = const.tile([128, NT], F32, tag="mt1")
    nc.sync.dma_start(out=mt1, in_=mt1_p[:])
    st = const.tile([128, 1], F32, tag="st")
    nc.sync.dma_start(out=st, in_=st_p[:])

    tiles = {}
    order = (
        [(lc, bh) for lc in range(LCH) for bh in range(HPC)]
        if INTERLEAVE
        else [(lc, bh) for bh in range(HPC) for lc in range(LCH)]
    )
    for bh in (range(HPC) if INTERLEAVE else []):
        qt = big.tile([128, L], F32R, tag="qt")
        nc.sync.dma_start(out=qt, in_=qt_p[bh])
        kt = big.tile([128, S], F32R, tag="kt")
        nc.sync.dma_start(out=kt, in_=kt_p[bh])
        ve = big.tile([128, NT, D + 1], F32R, tag="ve")
        nc.sync.dma_start(
            out=ve, in_=v_p[bh].rearrange("(t p) d -> p t d", p=128)
        )
        vn = big.tile([128, NT, D + 1], F32R, tag="vn")
        nc.sync.dma_start(
            out=vn, in_=vn_p[bh].rearrange("(t p) d -> p t d", p=128)
        )
        tiles[bh] = (qt, kt, ve, vn)
    for lc, bh in order:
        if not INTERLEAVE and lc == 0:
            qt = big.tile([128, L], F32R, tag="qt")
            kt = big.tile([128, S], F32R, tag="kt")
            if HALF_LOADS:
                nc.sync.dma_start(out=kt[0:64, :], in_=kt_p[bh, 0:64, :])
                nc.sync.dma_start(out=qt[0:64, :], in_=qt_p[bh, 0:64, :])
                nc.sync.dma_start(out=kt[64:128, :], in_=kt_p[bh, 64:128, :])
                nc.sync.dma_start(out=qt[64:128, :], in_=qt_p[bh, 64:128, :])
            else:
                nc.sync.dma_start(out=qt, in_=qt_p[bh])
                nc.sync.dma_start(out=kt, in_=kt_p[bh])
            ve = big.tile([128, NT, D + 1], F32R, tag="ve")
            nc.sync.dma_start(
                out=ve, in_=v_p[bh].rearrange("(t p) d -> p t d", p=128)
            )
            vn = big.tile([128, NT, D + 1], F32R, tag="vn")
            nc.sync.dma_start(
                out=vn, in_=vn_p[bh].rearrange("(t p) d -> p t d", p=128)
            )
            tiles[bh] = (qt, kt, ve, vn)
        qt, kt, ve, vn = tiles[bh]
        if True:
            lcb = 512 * lc
            oac_e = ps_oac.tile([D + 1, 512], F32, tag="oe")
            oac_n = ps_oac.tile([D + 1, 512], F32, tag="on")
            njs = 4 * lc + 4
            js_order = (
                list(range(njs - 1, -1, -1)) if JS_DESC else list(range(njs))
            )
            first_js = js_order[0]
            for js in js_order:
                off = max(0, 128 * js - lcb)
                sb = 128 * js
                lsl = slice(lcb + off, lcb + 512)
                pt_ps = ps_pt.tile([128, 2, 512], F32, tag="pt")
                nc.tensor.matmul(
                    pt_ps[:, 0, off:512],
                    kt[0:64, sb : sb + 128],
                    qt[0:64, lsl],
                    start=True, stop=True, tile_position=(0, 0),
                )
                nc.tensor.matmul(
                    pt_ps[:, 1, off:512],
                    kt[64:128, sb : sb + 128],
                    qt[64:128, lsl],
                    start=True, stop=True, tile_position=(64, 0),
                )
                pt_sb = pts.tile([128, 2, 512], F32R, tag="ptsb")
                nc.scalar.activation(
                    out=pt_sb[:, :, off:512],
                    in_=pt_ps[:, :, off:512],
                    func=EXPF,
                    bias=cdelta[:, js : js + 1],
                    scale=st,
                )
                if sb >= lcb:  # diagonal tile: mask l < s
                    _meng = nc.gpsimd if MASK_GPS else nc.vector
                    for br in range(2):
                        _meng.tensor_mul(
                            pt_sb[:, br, off : off + 128],
                            pt_sb[:, br, off : off + 128],
                            mask,
                        )
                last = js == js_order[-1]
                nc.tensor.matmul(
                    oac_e[:, off:512],
                    ve[:, js, :],
                    pt_sb[:, 0, off:512],
                    start=(js == first_js), stop=last,
                )
                nc.tensor.matmul(
                    oac_n[:, off:512],
                    vn[:, js, :],
                    pt_sb[:, 1, off:512],
                    start=(js == first_js), stop=last,
                )

            _ceng = nc.gpsimd if COPY_GPS else nc.vector
            oe_sb = osb.tile([D + 1, 512], F32, tag="oesb")
            _ceng.tensor_copy(out=oe_sb, in_=oac_e)
            on_sb = osb.tile([D + 1, 512], F32, tag="onsb")
            _ceng.tensor_copy(out=on_sb, in_=oac_n)
            for t4 in range(4):
                lt = 4 * lc + t4
                csl = slice(128 * t4, 128 * t4 + 128)
                tr_e = ps_tr.tile([128, 65], F32, tag="tr")
                nc.tensor.transpose(tr_e, oe_sb[:, csl], ident[0:65, 0:65])
                tr_n = ps_tr.tile([128, 65], F32, tag="tr")
                nc.tensor.transpose(tr_n, on_sb[:, csl], ident[0:65, 0:65])
                rec_e = sml.tile([128, 1], F32, tag="sml")
                nc.vector.reciprocal(rec_e, tr_e[:, 64:65])
                rec_n = sml.tile([128, 1], F32, tag="sml")
                nc.vector.reciprocal(rec_n, tr_n[:, 64:65])
                se = sml.tile([128, 1], F32, tag="sml")
                nc.vector.tensor_mul(se, rec_e, mt[:, lt : lt + 1])
                sn = sml.tile([128, 1], F32, tag="sml")
                nc.vector.tensor_mul(sn, rec_n, mt1[:, lt : lt + 1])
                obuf = ob.tile([128, D], F32, tag="ob")
                nc.vector.tensor_scalar_mul(obuf, tr_e[:, 0:64], se)
                nc.vector.scalar_tensor_tensor(
                    out=obuf, in0=tr_n[:, 0:64], scalar=sn, in1=obuf,
                    op0=MUL, op1=ADD,
                )
                nc.sync.dma_start(
                    out=out_p[bh, 128 * lt : 128 * lt + 128, :],
                    in_=obuf,
                )


def _host_in_maps(queries, keys, values, keys_null, values_null, tau, delta,
                  soft_mask):
    ident = np.eye(128, dtype=np.float32)
    mask = np.triu(np.ones((128, 128), dtype=np.float32))

    in_maps = []
    for c in range(NCORES):
        b, h0 = c // 4, HPC * (c % 4)
        qt = np.empty((HPC, 128, L), np.float32)
        kt = np.empty((HPC, 128, S), np.float32)
        v = np.empty((HPC, S, D + 1), np.float32)
        vn = np.empty((HPC, S, D + 1), np.float32)
        for bh in range(HPC):
            h = h0 + bh
            qT = queries[b, :, h, :].T  # [E, L]
            qt[bh, 0:64] = qT
            qt[bh, 64:128] = qT
            kt[bh, 0:64] = keys[b, :, h, :].T
            kt[bh, 64:128] = keys_null[b, :, h, :].T
            v[bh, :, 0:D] = values[b, :, h, :]
            v[bh, :, D] = 1.0
            vn[bh, :, 0:D] = values_null[b, :, h, :]
            vn[bh, :, D] = 1.0
        m_t = np.ascontiguousarray(soft_mask[b].reshape(NT, 128).T)
        in_maps.append(
            dict(
                qt=qt, kt=kt, v=v, vn=vn,
                st=np.full((128, 1), tau[b, 0] / 8.0, np.float32),
                cdelta=np.ascontiguousarray((delta[b] / 8.0).reshape(NT, 128).T),
                mt=m_t,
                mt1=np.ascontiguousarray(1.0 - m_t),
                ident=ident,
                mask=mask,
            )
        )
    return in_maps


def kernel(queries, keys, values, keys_null, values_null, tau, delta, soft_mask):
    global _NC, LAST_EXEC_NS
    queries = np.asarray(queries, dtype=np.float32)
    keys = np.asarray(keys, dtype=np.float32)
    values = np.asarray(values, dtype=np.float32)
    keys_null = np.asarray(keys_null, dtype=np.float32)
    values_null = np.asarray(values_null, dtype=np.float32)
    tau = np.asarray(tau, dtype=np.float32)
    delta = np.asarray(delta, dtype=np.float32)
    soft_mask = np.asarray(soft_mask, dtype=np.float32)

    if _NC is None:
        _NC = _build()

    in_maps = _host_in_maps(
        queries, keys, values, keys_null, values_null, tau, delta, soft_mask
    )
    res = run_bass_kernel_spmd(
        _NC, in_maps, core_ids=list(range(NCORES)), trace=TRACE
    )
    LAST_EXEC_NS = res.exec_time_ns

    out = np.empty((B, L, H, D), np.float32)
    for c in range(NCORES):
        b, h0 = c // 4, HPC * (c % 4)
        out[b, :, h0 : h0 + HPC, :] = res.results[c]["out"].transpose(1, 0, 2)
    return out



# revision 7
# speedup vs baseline: 2.0595x; 2.0595x over previous
"""DSAttention TRN2 Bass kernel.

Reference (per batch b, head h, branch):
    z[l,s] = (q[l]·k[s]) * tau[b]/8 + delta[b,s]/8        (causal: s <= l)
    A = softmax_s(z);  O = A @ V
    out = m*O_edit + (1-m)*O_null,  m = soft_mask[b,l]

Sharding: B*H = 16 (b,h) slices -> 8 cores x 2 heads. Same SPMD program on
every core; core c gets b = c//4, heads 2*(c%4), 2*(c%4)+1.

Per-core algorithm (transposed-score flash attention, bf16 inputs):
  - delta folded into V on the host: exp(z) = exp(qk*tau/8)*exp(delta_s/8),
    and the second factor scales row s of the AV contraction, so
    V'[s,:] = V[s,:]*exp(delta_s/8) with the denominator column exp(delta_s/8)
    replacing the ones column. The ACTIVATE then needs only scale=tau/8 and
    no per-tile bias.
  - host packs per head (bf16): qkt[128, {k,q}, L] where k-rows are
    [K_e^T; K_n^T] and q-rows [Q^T; Q^T] (dup), so the two branches' QK^T
    matmuls run row-packed (tile_position (0,0)/(64,0)); V' pre-swizzled to
    vv[128, {e,n}, NT, 65] so its DMA is straight contiguous descriptors.
  - flat (bh, lc, js) work stream with a one-item QK lookahead: the next
    score matmul is issued before the current AV pair, so the ACT engine
    (the bottleneck: exp of every causal score) never waits at tile/chunk
    boundaries.
  - scores^T tile [s:128, l:<=512] per (S-tile, L-chunk), trimmed to the
    causal region; exp on ACT with fused scale=tau/8, bf16 out; diagonal
    128x128 block masked post-exp for both branches in one DVE op
    (stride-0 broadcast of the triangular 0/1 tile).
  - O^T[65, 512] accumulates AV matmuls over S-tiles in PSUM (column 64 =
    softmax denominator).
  - epilogue per L-chunk: copy O^T to SBUF (bf16), PE-transpose the four
    128-blocks into one PSUM tile [128, 4, 65], batched reciprocal +
    soft-mask scaling ([128,4] ops), blend with stride-0 broadcast
    multiplies into a per-(b,h) staging tile, one DMA store per L-chunk.

REPEAT > 1 wraps the whole per-core program in a hardware For_i loop; used
by the timing harness to measure per-iteration HW time from wall-clock
deltas (transfers cancel).
"""

import contextlib

import numpy as np

import concourse.bass as bass
import concourse.tile as tile
from concourse import bacc, mybir
from concourse.bass_utils import run_bass_kernel_spmd

B, L, S, H, E, D = 2, 2048, 2048, 8, 64, 64
NCORES = 8
HPC = 2            # heads per core
NT = 16            # 128-row tiles in 2048
LCH = 4            # 512-wide L chunks
F32 = mybir.dt.float32
BF16 = mybir.dt.bfloat16
EXPF = mybir.ActivationFunctionType.Exp
MUL = mybir.AluOpType.mult
ADD = mybir.AluOpType.add

TRACE = False
LAST_EXEC_NS = None
PTS_BUFS = 4
REPEAT = 1

_NC = None


def _build():
    nc = bacc.Bacc("TRN2")
    qkt_p = nc.declare_dram_parameter("qkt", [HPC, 128, 2, L], BF16, isOutput=False)
    vv_p = nc.declare_dram_parameter("vv", [HPC, 128, 2, NT, D + 1], BF16,
                                     isOutput=False)
    st_p = nc.declare_dram_parameter("st", [128, 1], F32, isOutput=False)
    mts_p = nc.declare_dram_parameter("mts", [128, 2, NT], F32, isOutput=False)
    id_p = nc.declare_dram_parameter("ident", [65, 65], BF16, isOutput=False)
    mk_p = nc.declare_dram_parameter("mask", [128, 128], BF16, isOutput=False)
    out_p = nc.declare_dram_parameter("out", [HPC, L, D], F32, isOutput=True)
    params = (qkt_p, vv_p, st_p, mts_p, id_p, mk_p, out_p)

    with tile.TileContext(nc) as tc:
        with (
            tc.tile_pool(name="const", bufs=1) as const,
            tc.tile_pool(name="big", bufs=2) as big,
            tc.tile_pool(name="pts", bufs=PTS_BUFS) as pts,
            tc.tile_pool(name="osb", bufs=3) as osb,
            tc.tile_pool(name="sml", bufs=8) as sml,
            tc.tile_pool(name="stage", bufs=2) as stage,
            tc.tile_pool(name="ps_pt", bufs=2, space="PSUM") as ps_pt,
            tc.tile_pool(name="ps_oac", bufs=1, space="PSUM") as ps_oac,
            tc.tile_pool(name="ps_tr", bufs=1, space="PSUM") as ps_tr,
        ):
            pools = (const, big, pts, osb, sml, stage, ps_pt, ps_oac, ps_tr)
            rep = (
                tc.For_i(0, REPEAT, 1)
                if REPEAT > 1
                else contextlib.nullcontext()
            )
            with rep:
                _body(nc, pools, params)
    if not nc.is_finalized():
        nc.finalize()
    return nc


def _body(nc, pools, params):
    const, big, pts, osb, sml, stage, ps_pt, ps_oac, ps_tr = pools
    qkt_p, vv_p, st_p, mts_p, id_p, mk_p, out_p = params

    st = const.tile([128, 1], F32, tag="st")
    nc.sync.dma_start(out=st, in_=st_p[:])
    # warm the exp table while input DMAs stream
    warm = sml.tile([128, 1], F32, tag="sml")
    nc.scalar.activation(out=warm, in_=st, func=EXPF)

    # input loads up-front, in need-order; stores go last on this (SP) queue
    tiles = {}
    for bh in range(HPC):
        qkt = big.tile([128, 2, L], BF16, tag="qkt")
        vv = big.tile([128, 2, NT, D + 1], BF16, tag="vv")
        nc.sync.dma_start(out=qkt[:, :, 0:512], in_=qkt_p[bh, :, :, 0:512])
        if bh == 0:
            mask = const.tile([128, 128], BF16, tag="mask")
            nc.sync.dma_start(out=mask, in_=mk_p[:])
        nc.sync.dma_start(out=vv, in_=vv_p[bh])
        if bh == 0:
            ident = const.tile([65, 65], BF16, tag="ident")
            nc.sync.dma_start(out=ident, in_=id_p[:])
            mts = const.tile([128, 2, NT], F32, tag="mts")
            nc.sync.dma_start(out=mts, in_=mts_p[:])
        nc.sync.dma_start(out=qkt[:, :, 512:L], in_=qkt_p[bh, :, :, 512:L])
        tiles[bh] = (qkt, vv)

    work = [(bh, lc, js)
            for bh in range(HPC)
            for lc in range(LCH)
            for js in range(4 * lc + 4)]

    pt_of = {}

    def issue_qk(item):
        bh, lc, js = item
        qkt = tiles[bh][0]
        lcb = 512 * lc
        off = max(0, 128 * js - lcb)
        sb = 128 * js
        lsl = slice(lcb + off, lcb + 512)
        pt_ps = ps_pt.tile([128, 2, 512], F32, tag="pt")
        nc.tensor.matmul(
            pt_ps[:, 0, off:512],
            qkt[0:64, 0, sb : sb + 128],
            qkt[0:64, 1, lsl],
            start=True, stop=True, tile_position=(0, 0),
        )
        nc.tensor.matmul(
            pt_ps[:, 1, off:512],
            qkt[64:128, 0, sb : sb + 128],
            qkt[64:128, 1, lsl],
            start=True, stop=True, tile_position=(64, 0),
        )
        pt_of[item] = pt_ps

    issue_qk(work[0])
    oac = {}
    stg = {}
    for i, item in enumerate(work):
        bh, lc, js = item
        lcb = 512 * lc
        off = max(0, 128 * js - lcb)
        njs = 4 * lc + 4
        vv = tiles[bh][1]
        if lc == 0 and js == 0:
            stg[bh] = stage.tile([128, NT, D], F32, tag="stg", name="stg")
        if js == 0:
            oac[0] = ps_oac.tile([D + 1, 512], F32, tag="oe", name="oe")
            oac[1] = ps_oac.tile([D + 1, 512], F32, tag="on", name="on")

        pt_ps = pt_of.pop(item)
        pt_sb = pts.tile([128, 2, 512], BF16, tag="ptsb")
        nc.scalar.activation(
            out=pt_sb[:, :, off:512],
            in_=pt_ps[:, :, off:512],
            func=EXPF,
            scale=st,
        )
        if i + 1 < len(work):
            issue_qk(work[i + 1])
        if 128 * js >= lcb:  # diagonal tile: mask l < s, both branches at once
            mslc = pt_sb[:, :, off : off + 128]
            nc.vector.tensor_mul(
                mslc, mslc, mask.unsqueeze(1).broadcast_to([128, 2, 128])
            )
        last = js == njs - 1
        nc.tensor.matmul(
            oac[0][:, off:512],
            vv[:, 0, js, :],
            pt_sb[:, 0, off:512],
            start=(js == 0), stop=last,
        )
        nc.tensor.matmul(
            oac[1][:, off:512],
            vv[:, 1, js, :],
            pt_sb[:, 1, off:512],
            start=(js == 0), stop=last,
        )
        if not last:
            continue

        # epilogue for (bh, lc)
        oe_sb = osb.tile([D + 1, 512], BF16, tag="oesb")
        nc.vector.tensor_copy(out=oe_sb, in_=oac[0])
        on_sb = osb.tile([D + 1, 512], BF16, tag="onsb")
        nc.vector.tensor_copy(out=on_sb, in_=oac[1])
        tr_e = ps_tr.tile([128, 4, 66], BF16, tag="tre")
        tr_n = ps_tr.tile([128, 4, 66], BF16, tag="trn")
        for t4 in range(4):
            csl = slice(128 * t4, 128 * t4 + 128)
            nc.tensor.transpose(tr_e[:, t4, 0:65], oe_sb[:, csl], ident)
            nc.tensor.transpose(tr_n[:, t4, 0:65], on_sb[:, csl], ident)
        rec_e = sml.tile([128, 4], F32, tag="sml4")
        nc.vector.reciprocal(rec_e, tr_e[:, :, 64])
        rec_n = sml.tile([128, 4], F32, tag="sml4")
        nc.vector.reciprocal(rec_n, tr_n[:, :, 64])
        se = sml.tile([128, 4], F32, tag="sml4")
        nc.vector.tensor_mul(se, rec_e, mts[:, 0, 4 * lc : 4 * lc + 4])
        sn = sml.tile([128, 4], F32, tag="sml4")
        nc.vector.tensor_mul(sn, rec_n, mts[:, 1, 4 * lc : 4 * lc + 4])
        ssl = stg[bh][:, 4 * lc : 4 * lc + 4, :]
        tmp = sml.tile([128, 4, D], F32, tag="tmp")
        nc.vector.tensor_mul(
            ssl, tr_e[:, :, 0:64], se.unsqueeze(2).broadcast_to([128, 4, 64])
        )
        nc.vector.tensor_mul(
            tmp, tr_n[:, :, 0:64], sn.unsqueeze(2).broadcast_to([128, 4, 64])
        )
        nc.vector.tensor_add(ssl, ssl, tmp)
        nc.sync.dma_start(
            out=out_p[bh].rearrange("(t p) d -> p t d", p=128)[
                :, 4 * lc : 4 * lc + 4, :
            ],
            in_=ssl,
        )


def _host_in_maps(queries, keys, values, keys_null, values_null, tau, delta,
                  soft_mask):
    ident = np.eye(65, dtype=np.float32)
    mask = np.triu(np.ones((128, 128), dtype=np.float32))

    in_maps = []
    for c in range(NCORES):
        b, h0 = c // 4, HPC * (c % 4)
        w = np.exp(delta[b] / 8.0).astype(np.float32)  # [S]
        qkt = np.empty((HPC, 128, 2, L), np.float32)
        vv = np.empty((HPC, 2, S, D + 1), np.float32)
        for bh in range(HPC):
            h = h0 + bh
            qT = queries[b, :, h, :].T  # [E, L]
            qkt[bh, 0:64, 1] = qT
            qkt[bh, 64:128, 1] = qT
            qkt[bh, 0:64, 0] = keys[b, :, h, :].T
            qkt[bh, 64:128, 0] = keys_null[b, :, h, :].T
            vv[bh, 0, :, 0:D] = values[b, :, h, :] * w[:, None]
            vv[bh, 1, :, 0:D] = values_null[b, :, h, :] * w[:, None]
            vv[bh, :, :, D] = w
        # swizzle to [HPC, 128, 2, NT, D+1]: partition p holds rows s=t*128+p
        vsw = np.ascontiguousarray(
            vv.reshape(HPC, 2, NT, 128, D + 1).transpose(0, 3, 1, 2, 4))
        m_t = soft_mask[b].reshape(NT, 128).T  # [128, NT]
        mts = np.ascontiguousarray(
            np.stack([m_t, 1.0 - m_t], axis=1))  # [128, 2, NT]
        in_maps.append(
            dict(
                qkt=_bf16(qkt), vv=_bf16(vsw),
                st=np.full((128, 1), tau[b, 0] / 8.0, np.float32),
                mts=mts,
                ident=_bf16(ident),
                mask=_bf16(mask),
            )
        )
    return in_maps


def _bf16(x):
    import jax.numpy as jnp
    return np.asarray(jnp.asarray(x, jnp.bfloat16))


def kernel(queries, keys, values, keys_null, values_null, tau, delta, soft_mask):
    global _NC, LAST_EXEC_NS
    queries = np.asarray(queries, dtype=np.float32)
    keys = np.asarray(keys, dtype=np.float32)
    values = np.asarray(values, dtype=np.float32)
    keys_null = np.asarray(keys_null, dtype=np.float32)
    values_null = np.asarray(values_null, dtype=np.float32)
    tau = np.asarray(tau, dtype=np.float32)
    delta = np.asarray(delta, dtype=np.float32)
    soft_mask = np.asarray(soft_mask, dtype=np.float32)

    if _NC is None:
        _NC = _build()

    in_maps = _host_in_maps(
        queries, keys, values, keys_null, values_null, tau, delta, soft_mask
    )
    res = run_bass_kernel_spmd(
        _NC, in_maps, core_ids=list(range(NCORES)), trace=TRACE
    )
    LAST_EXEC_NS = res.exec_time_ns

    out = np.empty((B, L, H, D), np.float32)
    for c in range(NCORES):
        b, h0 = c // 4, HPC * (c % 4)
        out[b, :, h0 : h0 + HPC, :] = res.results[c]["out"].transpose(1, 0, 2)
    return out
